# revision 16
# baseline (speedup 1.0000x reference)
"""ChannelTimeAttention Trainium2 kernel (v3: contiguous DMA + lean compute).

Reference computation (per (b, c) pair, all independent):
    pooled = AdaptiveAvgPool(x[b, :, c]) -> [t, 8*8]      (7x7 block means)
    q = pooled @ Wq + bq ; k = pooled @ Wk + bk           [t, 32]
    att = softmax(q @ k.T / sqrt(t))                      [t, t]
    out[b, :, c] = att @ x[b, :, c].reshape(t, h*w)

Sharding: data-parallel over b — one batch element per NeuronCore (8 cores).

DMA layout: partition i = t*8 + cg (cg = c//8), free = cl*hw + hw_idx
(cl = c%8).  Every descriptor is a contiguous >=25KB DRAM run per
partition row — measured 414 GB/s chained-quarter reads vs ~180 GB/s for
a strided-pack pattern.  x streams in as 4 column-quarter DMAs chained
on the ACT HWDGE ring (input loads carry no sem-waits, so they dispatch
immediately); out streams as 4 quarter DMAs on the SP HWDGE ring
(dedicated, so compute-dependent waits never stall the input chain).
x/v are declared float32r end-to-end: the DMA is a same-dtype copy and
the BIR verifier accepts it as a rounded FP32r matmul operand; pooling
reads v bitcast back to f32.

Compute per slot j = 0..7 (channel c = cg*8 + j):
  DVE  single fused reduce over both 7-blocks -> pooled [128, 8*8]
  PE   transpose(pooled) -> pooledT (ACT-evac to bf16) [64, 128]
  PE   q^T, k^T (bf16 weights); ACT bias-add into kA=[k;A], qB=[q;B]
       where A/B are 8 indicator rows encoding the block-diagonal mask
       as a rank-8 term: kA.T@qB = scores^T + mask, qB.T@kA = scores + mask
  ACT  exp(PSUM) -> eT (f32r, the stage-2 stationary operand); second
       exp with accum_out -> softmax denominator; DVE reciprocal.
       No max-subtraction: folded weights keep |scores| ~ 1e-5.
  PE   out_unnorm = eT.T @ v in 7 N=448 f32r chunks; PSUM evacuation
       applies 1/denominator as a per-partition scale (DVE/ACT split)
1/49 (pool mean) and 1/sqrt(16) (score scale) fold into Wq/bq/Wk host-side.
"""

import numpy as np

B, T, C, H, W = 8, 16, 64, 56, 56
DS = 8
DIN = DS * DS  # 64
DOUT = 32
HW = H * W  # 3136
P = 128
NQ = 4  # input/output column-quarter DMAs
NCH = 7  # output free-dim chunks per slot
CHN = HW // NCH  # 448
N_CORES = 8
MASK_NEG = -30.0
CW = 164  # consts width (f32 cols)


def _build_nc():
    import concourse.bacc as bacc
    import concourse.tile as tile
    from concourse import mybir
    from concourse.masks import make_identity
    from contextlib import ExitStack

    f32 = mybir.dt.float32
    f32r = mybir.dt.float32r
    bf16 = mybir.dt.bfloat16
    nc = bacc.Bacc(trn_type="TRN2", num_swdge_queues=2)

    x_h = nc.dram_tensor("x", [T, C, H, W], f32r, kind="ExternalInput")
    # consts [128, 164] f32: cols 0:16 wq-bf16(rows 0:64), 16:32 wk-bf16,
    # col 32 bq (rows 0:32), col 33 bk, cols 34:98 A-bf16 (rows 32:40),
    # cols 98:162 B-bf16 (rows 32:40)
    cn_h = nc.dram_tensor("consts", [P, CW], f32, kind="ExternalInput")
    out_h = nc.dram_tensor("out", [T, C, H, W], f32, kind="ExternalOutput")

    XY = mybir.AxisListType.XY
    Exp = mybir.ActivationFunctionType.Exp

    with ExitStack() as ctx:
        tc = ctx.enter_context(tile.TileContext(nc))
        singles = ctx.enter_context(tc.tile_pool(name="singles", bufs=1))
        opool = ctx.enter_context(tc.tile_pool(name="opool", bufs=3))
        small = ctx.enter_context(tc.tile_pool(name="small", bufs=3))
        psA = ctx.enter_context(tc.tile_pool(name="psA", bufs=1, space="PSUM"))
        psB = ctx.enter_context(tc.tile_pool(name="psB", bufs=3, space="PSUM"))

        consts = singles.tile([P, CW], f32)
        nc.scalar.dma_start(out=consts, in_=cn_h[:])
        wq_b = consts[0:DIN, 0:16].bitcast(bf16)  # [64, 32]
        wk_b = consts[0:DIN, 16:32].bitcast(bf16)
        bq = consts[0:DOUT, 32:33]
        bk = consts[0:DOUT, 33:34]
        A_b = consts[32:40, 34:98].bitcast(bf16)  # [8, 128]
        B_b = consts[32:40, 98:162].bitcast(bf16)
        ident = singles.tile([P, P], f32)
        make_identity(nc, ident[:])

        src = x_h[:].rearrange(
            "t (cg cl4 cl) h w -> cl4 (t cg) (cl h w)", cg=8, cl4=NQ, cl=2
        )
        dst = out_h[:].rearrange(
            "t (cg cl4 cl) h w -> cl4 (t cg) (cl h w)", cg=8, cl4=NQ, cl=2
        )

        # tile_wait_until: scheduling hint only — tells the Tile scheduler's
        # cost model that the quarters CHAIN on one ring (arrive ~9us apart)
        # instead of landing in parallel.  Without it the scheduler thinks
        # later quarters arrive early and head-of-line-blocks PE behind
        # pool-dependent work of not-yet-arrived quarters.
        v_tiles = []
        for q in range(NQ):
            v = singles.tile([P, 2 * HW], f32r, tag=f"v{q}", name=f"v{q}")
            with tc.tile_wait_until(0.008 + 0.009 * q):
                nc.scalar.dma_start(out=v[:], in_=src[q])
            v_tiles.append(v)

        o_tiles = {}
        stage2 = []

        def emit_stage1(j):
            q, u = j // 2, j % 2
            v = v_tiles[q]

            # ---- adaptive avg pool: one fused reduce over (7h x 7w) ----
            pooled = small.tile([P, DS, DS], f32, tag="pooled")
            nc.vector.reduce_sum(
                out=pooled[:],
                in_=v[:, u * HW : (u + 1) * HW]
                .bitcast(f32)
                .rearrange(
                    "p (i u2 j vv) -> p i j u2 vv", i=DS, u2=7, j=DS, vv=7
                ),
                axis=XY,
            )

            # ---- pooled^T so the q/k matmuls contract over d_in ----
            pT_ps = psA.tile([DIN, P], f32, tag="pT")
            nc.tensor.transpose(
                pT_ps, pooled[:].rearrange("p i j -> p (i j)"), ident
            )
            pooledT = small.tile([DIN, P], bf16, tag="pooledT")
            nc.scalar.copy(pooledT, pT_ps)

            # ---- q^T, k^T [32, 128]; bias lands during ACT evacuation into
            # [40, 128] tiles whose extra 8 rows hold the mask factors ----
            q_ps = psA.tile([DOUT, P], f32, tag="q")
            nc.tensor.matmul(q_ps, lhsT=wq_b, rhs=pooledT, start=True, stop=True)
            k_ps = psA.tile([DOUT, P], f32, tag="k")
            nc.tensor.matmul(k_ps, lhsT=wk_b, rhs=pooledT, start=True, stop=True)
            qB = small.tile([40, P], bf16, tag="qB")
            kA = small.tile([40, P], bf16, tag="kA")
            nc.scalar.add(qB[0:DOUT, :], q_ps, bq)
            nc.scalar.add(kA[0:DOUT, :], k_ps, bk)
            nc.scalar.copy(qB[DOUT:40, :], B_b)
            nc.scalar.copy(kA[DOUT:40, :], A_b)

            # ---- masked scores both ways (rank-8 mask inside the matmul) --
            scT_ps = psA.tile([P, P], f32, tag="scT")
            nc.tensor.matmul(scT_ps, lhsT=kA[:], rhs=qB[:], start=True, stop=True)
            sc_ps = psA.tile([P, P], f32, tag="sc")
            nc.tensor.matmul(sc_ps, lhsT=qB[:], rhs=kA[:], start=True, stop=True)

            eT = small.tile([P, P], f32r, tag="eT")
            nc.scalar.activation(out=eT, in_=scT_ps, func=Exp)
            edump = small.tile([P, P], f32, tag="edump")
            ssum = small.tile([P, 1], f32, tag="ssum")
            nc.scalar.activation(out=edump, in_=sc_ps, func=Exp, accum_out=ssum)
            stage2.append((j, eT, ssum))

        def emit_stage2(j, eT, ssum):
            q, u = j // 2, j % 2
            v = v_tiles[q]
            rinv = small.tile([P, 1], f32, tag="rinv")
            nc.vector.reciprocal(rinv, ssum)
            if u == 0:
                o_tiles[q] = opool.tile([P, 2 * HW], f32, tag="o", name="o")
                # claim the o slot with a cheap DVE op: it absorbs the WAR
                # wait on the out-DMA that previously read this slot
                nc.vector.memset(o_tiles[q][:, 0:1], 0.0)
            o = o_tiles[q]
            for ch in range(NCH):
                sl = slice(u * HW + ch * CHN, u * HW + (ch + 1) * CHN)
                ops = psB.tile([P, CHN], f32, tag="och")
                nc.tensor.matmul(
                    ops, lhsT=eT[:], rhs=v[:, sl], start=True, stop=True
                )
                # normalization folded into PSUM evacuation, split DVE/ACT
                if ch % 3 == 0:
                    nc.vector.tensor_scalar_mul(
                        out=o[:, sl], in0=ops, scalar1=rinv
                    )
                else:
                    nc.scalar.mul(o[:, sl], ops, rinv)
            if u == 1:
                nc.sync.dma_start(out=dst[q], in_=o[:])

        for j in range(2 * NQ):
            if j >= 1:
                emit_stage2(*stage2[j - 1])
            emit_stage1(j)
        emit_stage2(*stage2[2 * NQ - 1])

    nc.compile()
    return nc


def _pack_bf16(a):
    """Pack a [r, c] f32-precision array as bf16 pairs into [r, c//2] f32."""
    import ml_dtypes

    u16 = a.astype(ml_dtypes.bfloat16).view(np.uint16)
    u32 = u16[:, 0::2].astype(np.uint32) | (
        u16[:, 1::2].astype(np.uint32) << 16
    )
    return u32.view(np.float32)


def _host_consts(Wq, bq, Wk, bk):
    # fold pool-mean 1/49 into both weight mats; fold score 1/sqrt(t)=1/4
    # into the q side (weights AND bias)
    wq_eff = (Wq / (49.0 * 4.0)).astype(np.float32)
    bq_eff = (bq / 4.0).astype(np.float32)
    wk_eff = (Wk / 49.0).astype(np.float32)
    bk_eff = bk.astype(np.float32)
    consts = np.zeros((P, CW), dtype=np.float32)
    consts[0:DIN, 0:16] = _pack_bf16(wq_eff)
    consts[0:DIN, 16:32] = _pack_bf16(wk_eff)
    consts[0:DOUT, 32] = bq_eff
    consts[0:DOUT, 33] = bk_eff
    # rank-8 mask factors: (A.T @ B)[s, t] = MASK_NEG * (s%8 != t%8)
    r = np.arange(8)[:, None]
    s = np.arange(P)[None, :]
    A = (s % 8 == r).astype(np.float32)  # [8, 128]
    Bm = MASK_NEG * (1.0 - A)  # [8, 128]
    consts[32:40, 34:98] = _pack_bf16(A)
    consts[32:40, 98:162] = _pack_bf16(Bm)
    return consts


def kernel(x, Wq, bq, Wk, bk):
    from concourse.bass_utils import run_bass_kernel_spmd

    x = np.ascontiguousarray(x, dtype=np.float32)
    consts = _host_consts(Wq, bq, Wk, bk)

    nc = _build_nc()
    in_maps = [{"x": x[i], "consts": consts} for i in range(N_CORES)]
    res = run_bass_kernel_spmd(nc, in_maps, core_ids=list(range(N_CORES)))
    global LAST_RUN
    LAST_RUN = res
    out = np.stack([r["out"] for r in res.results], axis=0)
    return out


LAST_RUN = None
